# revision 8
# baseline (speedup 1.0000x reference)
"""Causal single-head attention on 8 Trainium2 NeuronCores (Bass/Tile).

Problem: X[4,4096,512] fp32, Wq/Wk/Wv[512,64] fp32.
  Q=XWq, K=XWk, V=XWv ; Z = softmax(mask(QK^T)/8) V    -> [4,4096,64]

Sharding (2 cores per batch, fully uniform SPMD program):
  - Keys/values are split by PARITY of 128-row key blocks: core A of a pair
    owns even key blocks, core B odd ones.  The host packs each core's key
    blocks contiguously, so both cores run the *identical* instruction
    stream on different data.
  - Each core computes, for every query tile, partial attention over its
    own half of the keys with un-normalized softmax (no max subtraction --
    logits here are ~N(0, 0.2^2) so exp never overflows):
        numerator   N_c = sum_k exp(s)*V,   denominator D_c = sum_k exp(s)
    The host combines  Z = (N_A + N_B) / (D_A + D_B)  exactly.
  - Denominators come for free as column 64 of V_ext = [V | 1] in the
    P^T @ V_ext matmul.
  - Causality at 128-block granularity is structural (k-block count grows
    with the query tile); the diagonal partial blocks are handled by
    multiplying exp(S) with a mask slice.  The mask is per-core INPUT DATA,
    which absorbs the even/odd key-parity difference between cores.

On-chip dataflow (all matmuls bf16 with fp32 PSUM accumulation):
  - scores are computed transposed  S^T[k,q] = K^T-block-stationary @ Q^T
    so that P^T = exp(S^T) feeds the PV matmul with no on-chip transpose.
  - Q^T and K^T are produced doubled across the partition dim ([W|W]
    weights) so score matmuls (contraction=64) run 2x packed in the PE
    array via row groups (partitions 0-63 / 64-127).
  - V is produced in natural [k,64] layout by making the X^T chunk the
    stationary operand.
  - DMAs are split/ordered by first consumption so the PE starts ~11us in;
    exp groups span 3 PSUM banks to amortize ACT instruction overhead.
"""

import numpy as np
import ml_dtypes

import concourse.bacc as bacc
import concourse.bass as bass
import concourse.mybir as mybir
import concourse.tile as tile

B, S, DIN, E = 4, 4096, 512, 64
PB = 128            # partition / key block
QT = 512            # query tile width
NQT = S // QT       # 8 query tiles
NKB = S // PB       # 32 key blocks per batch
HKB = NKB // 2      # 16 packed key blocks per core
SH = S // 2         # 2048 packed keys per core
NCORES = 8
SCALE = 1.0 / np.sqrt(E)
G = 3               # exp group: PSUM banks per score group

BF16 = ml_dtypes.bfloat16
BF = mybir.dt.bfloat16
F32 = mybir.dt.float32

_CACHE = {}


def _build():
    nc = bacc.Bacc("TRN2", target_bir_lowering=False, debug=False,
                   enable_asserts=False, num_devices=NCORES)

    xtf_h = nc.dram_tensor("xtf", [DIN, S], BF, kind="ExternalInput")
    xtk_h = nc.dram_tensor("xtk", [DIN, SH], BF, kind="ExternalInput")
    wq2_h = nc.dram_tensor("wq2", [DIN, 2 * E], BF, kind="ExternalInput")
    wk2_h = nc.dram_tensor("wk2", [DIN, 2 * E], BF, kind="ExternalInput")
    wv1_h = nc.dram_tensor("wv1", [DIN, E], BF, kind="ExternalInput")
    msk_h = nc.dram_tensor("msk", [PB, 896], BF, kind="ExternalInput")
    zt_h = nc.dram_tensor("zt", [E + 1, S], F32, kind="ExternalOutput")

    # [d_in, s] DRAM views reshaped to chunk form [p=128, c=4, s]
    xtf_r = xtf_h.ap().rearrange("(c p) s -> p c s", p=PB)
    xtk_r = xtk_h.ap().rearrange("(c p) s -> p c s", p=PB)
    zt = zt_h.ap()

    with tile.TileContext(nc) as tc:
        with (
            tc.tile_pool(name="big", bufs=1) as big,
            tc.tile_pool(name="pt", bufs=3) as ptp,
            tc.tile_pool(name="zsb", bufs=2) as zsbp,
            tc.tile_pool(name="spsum", bufs=2, space="PSUM") as sp,
            tc.tile_pool(name="zpsum", bufs=2, space="PSUM") as zp,
        ):
            # ---- persistent SBUF buffers ----
            xtf_sb = big.tile([PB, 4, S], BF, tag="xtf")
            xtk_sb = big.tile([PB, 4, SH], BF, tag="xtk")
            wq2_sb = big.tile([PB, 4, 2 * E], BF, tag="wq2")
            wk2_sb = big.tile([PB, 4, 2 * E], BF, tag="wk2")
            wv1_sb = big.tile([PB, 4, E], BF, tag="wv1")
            msk_sb = big.tile([PB, 896], BF, tag="msk")
            qt2 = big.tile([PB, S], BF, tag="qt2")      # doubled Q^T
            kt2 = big.tile([PB, SH], BF, tag="kt2")     # doubled K^T
            vext = big.tile([PB, HKB * (E + 1)], BF, tag="vext")

            dma = nc.sync.dma_start

            # ---- input DMAs, ordered by first consumption ----
            dma(wk2_sb[:], wk2_h.ap().rearrange("(c p) m -> p c m", p=PB))
            dma(wq2_sb[:], wq2_h.ap().rearrange("(c p) m -> p c m", p=PB))
            dma(wv1_sb[:], wv1_h.ap().rearrange("(c p) m -> p c m", p=PB))
            dma(msk_sb[:], msk_h.ap())
            dma(xtk_sb[:, :, 0:SH // 2], xtk_r[:, :, 0:SH // 2])
            dma(xtf_sb[:, :, 0:QT], xtf_r[:, :, 0:QT])
            dma(xtk_sb[:, :, SH // 2:SH], xtk_r[:, :, SH // 2:SH])
            dma(xtf_sb[:, :, QT:3 * QT], xtf_r[:, :, QT:3 * QT])
            dma(xtf_sb[:, :, 3 * QT:6 * QT], xtf_r[:, :, 3 * QT:6 * QT])
            dma(xtf_sb[:, :, 6 * QT:8 * QT], xtf_r[:, :, 6 * QT:8 * QT])

            # ones columns of V_ext (V blocks overwrite cols 0..63 later)
            nc.vector.memset(vext[:], 1.0)

            def k_proj(s4):
                k_ps = zp.tile([PB, QT], F32, tag="zp", name="k_ps")
                for c in range(4):
                    nc.tensor.matmul(
                        k_ps[:], wk2_sb[:, c, :],
                        xtk_sb[:, c, QT * s4:QT * (s4 + 1)],
                        start=(c == 0), stop=(c == 3))
                nc.vector.tensor_copy(kt2[:, QT * s4:QT * (s4 + 1)], k_ps[:])

            def v_proj(j):
                v_ps = zp.tile([PB, QT], F32, tag="zp", name="v_ps")
                for c in range(4):
                    nc.tensor.matmul(
                        v_ps[:, 0:E], xtk_sb[:, c, PB * j:PB * (j + 1)],
                        wv1_sb[:, c, :],
                        start=(c == 0), stop=(c == 3))
                nc.vector.tensor_copy(
                    vext[:, (E + 1) * j:(E + 1) * j + E], v_ps[:, 0:E])

            def q_proj(t):
                q_ps = zp.tile([PB, QT], F32, tag="zp", name="q_ps")
                for c in range(4):
                    nc.tensor.matmul(
                        q_ps[:], wq2_sb[:, c, :],
                        xtf_sb[:, c, QT * t:QT * (t + 1)],
                        start=(c == 0), stop=(c == 3))
                nc.vector.tensor_copy(qt2[:, QT * t:QT * (t + 1)], q_ps[:])

            # ---- main loop over query tiles ----
            pend = None     # deferred PV group (to keep PE off ACT's tail)
            for t in range(NQT):
                if t % 2 == 0:
                    k_proj(t // 2)
                q_proj(t)
                v_proj(2 * t)
                v_proj(2 * t + 1)

                z_ps = zp.tile([E + 1, QT], F32, tag="zp", name="z_ps")
                njb = 2 * t + 2
                groups = [list(range(g, min(g + G, njb)))
                          for g in range(0, njb, G)]
                for gi, js in enumerate(groups):
                    s_ps = sp.tile([PB, G * QT], F32, tag="s", name="s_ps")
                    for j in js:
                        sl = j - js[0]
                        half = slice(0, 64) if j % 2 == 0 else slice(64, 128)
                        nc.tensor.matmul(
                            s_ps[:, QT * sl:QT * (sl + 1)],
                            kt2[half, PB * j:PB * (j + 1)],
                            qt2[half, QT * t:QT * (t + 1)],
                            start=True, stop=True)

                    # flush previous group's deferred PV matmuls
                    if pend is not None:
                        _flush_pv(nc, pend)
                        pend = None

                    w = QT * len(js)
                    pt = ptp.tile([PB, G * QT], BF, tag="pt", name="pt")
                    nc.scalar.activation(pt[:, 0:w], s_ps[:, 0:w],
                                         mybir.ActivationFunctionType.Exp,
                                         scale=float(SCALE))
                    for j in js:
                        if j >= 2 * t:   # diagonal blocks: causal masks
                            sl = j - js[0]
                            mo = 384 if j == 2 * t else 128
                            nc.vector.tensor_mul(
                                pt[:, QT * sl:QT * (sl + 1)],
                                pt[:, QT * sl:QT * (sl + 1)],
                                msk_sb[:, mo:mo + QT])
                    pend = (z_ps, vext, pt, js, t)

                # attach Z evacuation of this tile to the last deferred group
                pend = pend + (zt, zsbp)

            # tail: flush last tile's PV + evacuation
            _flush_pv(nc, pend)

    nc.compile()
    return nc


def _flush_pv(nc, pend):
    """Emit the deferred PV matmul group (and Z evacuation if attached)."""
    z_ps, vext, pt, js, t = pend[:5]
    for j in js:
        sl = j - js[0]
        nc.tensor.matmul(
            z_ps[:],
            vext[:, (E + 1) * j:(E + 1) * (j + 1)],
            pt[:, QT * sl:QT * (sl + 1)],
            start=(j == 0), stop=(j == 2 * t + 1))
    if len(pend) > 5:
        zt, zsbp = pend[5], pend[6]
        z_sb = zsbp.tile([E + 1, QT], F32, tag="zsb", name="z_sb")
        nc.vector.tensor_copy(z_sb[:], z_ps[:])
        nc.sync.dma_start(zt[:, QT * t:QT * (t + 1)], z_sb[:])


def _get_nc():
    if "nc" not in _CACHE:
        _CACHE["nc"] = _build()
    return _CACHE["nc"]


def _host_inputs(X, Wq, Wk, Wv):
    """Per-core input maps. Core 2b = parity 0 (even key blocks) of batch b,
    core 2b+1 = parity 1."""
    w2 = lambda w: np.concatenate([w, w], axis=1).astype(BF16)
    wq2, wk2 = w2(Wq), w2(Wk)
    wv1 = Wv.astype(BF16)
    # mask master: msk[i, u] = 1 if i <= u - 384 - 128*c
    u = np.arange(896)[None, :]
    i = np.arange(PB)[:, None]
    masks = [(i <= u - 384 - 128 * c).astype(BF16) for c in (0, 1)]

    in_maps = []
    for b in range(B):
        xb = np.asarray(X[b])
        xt = np.ascontiguousarray(xb.T).astype(BF16)              # [512,4096]
        xr = xb.reshape(NKB, PB, DIN)
        for c in (0, 1):
            xkv = np.ascontiguousarray(
                xr[c::2].reshape(SH, DIN).T).astype(BF16)          # [512,2048]
            in_maps.append({
                "xtf": xt, "xtk": xkv,
                "wq2": wq2, "wk2": wk2, "wv1": wv1,
                "msk": masks[c],
            })
    return in_maps


def _combine(results):
    Z = np.empty((B, S, E), np.float32)
    for b in range(B):
        za = results[2 * b]["zt"].astype(np.float32)
        zb = results[2 * b + 1]["zt"].astype(np.float32)
        num = za[:E] + zb[:E]
        den = za[E] + zb[E]
        Z[b] = (num / den[None, :]).T
    return Z


def kernel(X, Wq, Wk, Wv, _trace=False, _tmpdir=None):
    from concourse.bass_utils import run_bass_kernel_spmd
    nc = _get_nc()
    in_maps = _host_inputs(X, Wq, Wk, Wv)
    kw = {}
    if _tmpdir is not None:
        kw["tmpdir"] = _tmpdir
    res = run_bass_kernel_spmd(nc, in_maps, core_ids=list(range(NCORES)),
                               trace=_trace, **kw)
    _CACHE["last"] = res
    return _combine(res.results)


# revision 10
# speedup vs baseline: 1.1458x; 1.1458x over previous
"""Causal single-head attention on 8 Trainium2 NeuronCores (Bass/Tile).

Problem: X[4,4096,512] fp32, Wq/Wk/Wv[512,64] fp32.
  Q=XWq, K=XWk, V=XWv ; Z = softmax(mask(QK^T)/8) V    -> [4,4096,64]

Sharding: 2 cores per batch, fully uniform SPMD program.
  - Keys/values are split by PARITY of 128-row key blocks: core A of a pair
    owns even key blocks, core B odd ones.  Each core's X^T input is
    ROTATED left by 128*parity columns by the host, which makes "my key
    blocks" sit at even 128-col positions for BOTH cores -- so one
    instruction stream with static addresses serves both.
  - Each core computes, for every query tile, partial attention over its
    own half of the keys with un-normalized softmax (no max subtraction --
    logits here are ~N(0, 0.2^2) so exp cannot overflow):
        numerator   N_c = sum_k exp(s)*V,   denominator D_c = sum_k exp(s)
    The host combines  Z = (N_A + N_B) / (D_A + D_B)  exactly.  The
    rotation wraps one query block on core B (tile 7); the host simply
    uses A-only partials for those 128 queries (A covers them fully).
  - Denominators come for free as column 64 of V_ext = [V | 1] in the
    P^T @ V_ext matmul.
  - Causality at 128-block granularity is structural (k-block count grows
    with the query tile); diagonal blocks are fixed by multiplying exp(S)
    by one of two static triangular masks (rotation makes the needed mask
    content identical on both cores).

On-chip dataflow (all matmuls bf16, fp32 PSUM accumulation):
  - scores are computed transposed  S^T[k,q] = K^T-block-stationary @ Q^T
    so P^T = exp(S^T) feeds the PV matmul with no on-chip transpose.
  - Q^T and K^T are produced doubled across the partition dim ([W|W]
    weights) so score matmuls (contraction=64) run 2x packed in the PE
    array via row groups (partitions 0-63 / 64-127).
  - V is produced in natural [k,64] layout by making the X^T chunk the
    stationary operand; K projection reads even 128-col blocks of X^T via
    a strided access pattern.
  - DMAs are split and ordered by first consumption; the PE starts ~11us
    in and the first exp fires ~14us in.
"""

import numpy as np
import ml_dtypes

import concourse.bacc as bacc
import concourse.bass as bass
import concourse.mybir as mybir
import concourse.tile as tile

B, S, DIN, E = 4, 4096, 512, 64
PB = 128            # partition / key block
QT = 512            # query tile width
NQT = S // QT       # 8 query tiles
NKB = S // PB       # 32 key blocks per batch
HKB = NKB // 2      # 16 packed key blocks per core
SH = S // 2         # 2048 packed keys per core
NCORES = 8
SCALE = 1.0 / np.sqrt(E)
GJ = 2              # k-blocks per exp group (PSUM banks = GJ)

BF16 = ml_dtypes.bfloat16
BF = mybir.dt.bfloat16
F32 = mybir.dt.float32

_CACHE = {}


def _build():
    nc = bacc.Bacc("TRN2", target_bir_lowering=False, debug=False,
                   enable_asserts=False, num_devices=NCORES)

    xtf_h = nc.dram_tensor("xtf", [DIN, S], BF, kind="ExternalInput")
    wq2_h = nc.dram_tensor("wq2", [DIN, 2 * E], BF, kind="ExternalInput")
    wk2_h = nc.dram_tensor("wk2", [DIN, 2 * E], BF, kind="ExternalInput")
    wv1_h = nc.dram_tensor("wv1", [DIN, E], BF, kind="ExternalInput")
    msk_h = nc.dram_tensor("msk", [PB, 896], BF, kind="ExternalInput")
    zt_h = nc.dram_tensor("zt", [E + 1, S], F32, kind="ExternalOutput")

    xtf_r = xtf_h.ap().rearrange("(c p) s -> p c s", p=PB)
    zt = zt_h.ap()

    with tile.TileContext(nc) as tc:
        with (
            tc.tile_pool(name="big", bufs=1) as big,
            tc.tile_pool(name="pt", bufs=3) as ptp,
            tc.tile_pool(name="zsb", bufs=2) as zsbp,
            tc.tile_pool(name="ppsum", bufs=2, space="PSUM") as pp,
            tc.tile_pool(name="spsum", bufs=2, space="PSUM") as sp,
            tc.tile_pool(name="zpsum", bufs=2, space="PSUM") as zp,
        ):
            # ---- persistent SBUF buffers ----
            xtf_sb = big.tile([PB, 4, S], BF, tag="xtf")
            wq2_sb = big.tile([PB, 4, 2 * E], BF, tag="wq2")
            wk2_sb = big.tile([PB, 4, 2 * E], BF, tag="wk2")
            wv1_sb = big.tile([PB, 4, E], BF, tag="wv1")
            msk_sb = big.tile([PB, 896], BF, tag="msk")
            qt2 = big.tile([PB, S], BF, tag="qt2")      # doubled Q^T (rot)
            kt2 = big.tile([PB, SH], BF, tag="kt2")     # doubled K^T (packed)
            vext = big.tile([PB, HKB * (E + 1)], BF, tag="vext")

            dma = nc.sync.dma_start

            # ---- input DMAs, ordered by first consumption ----
            dma(wk2_sb[:], wk2_h.ap().rearrange("(c p) m -> p c m", p=PB))
            dma(wq2_sb[:], wq2_h.ap().rearrange("(c p) m -> p c m", p=PB))
            dma(wv1_sb[:], wv1_h.ap().rearrange("(c p) m -> p c m", p=PB))
            dma(msk_sb[:], msk_h.ap())
            for pc in range(4):     # 4 x 1 MB pieces of X^T
                lo, hi = 2 * QT * pc, 2 * QT * (pc + 1)
                dma(xtf_sb[:, :, lo:hi], xtf_r[:, :, lo:hi])

            # ones columns of V_ext (V blocks overwrite cols 0..63 later)
            nc.vector.memset(vext[:], 1.0)

            def even_blocks(ap2d, s4):
                """[128, 512] strided view: even 128-col blocks
                {8s4, 8s4+2, 8s4+4, 8s4+6} of a [128, S] AP."""
                seg = ap2d[:, 1024 * s4:1024 * (s4 + 1)]
                return seg.rearrange("p (b two x) -> p b two x",
                                     two=2, x=PB)[:, :, 0, :]

            def k_proj(s4):
                k_ps = pp.tile([PB, QT], F32, tag="proj", name="k_ps")
                for c in range(4):
                    nc.tensor.matmul(
                        k_ps[:], wk2_sb[:, c, :],
                        even_blocks(xtf_sb[:, c, :], s4),
                        start=(c == 0), stop=(c == 3))
                nc.vector.tensor_copy(kt2[:, QT * s4:QT * (s4 + 1)], k_ps[:])

            def v_proj(j):
                v_ps = pp.tile([PB, QT], F32, tag="proj", name="v_ps")
                for c in range(4):
                    nc.tensor.matmul(
                        v_ps[:, 0:E], xtf_sb[:, c, 2 * PB * j:2 * PB * j + PB],
                        wv1_sb[:, c, :],
                        start=(c == 0), stop=(c == 3))
                nc.vector.tensor_copy(
                    vext[:, (E + 1) * j:(E + 1) * j + E], v_ps[:, 0:E])

            def q_proj(t):
                q_ps = pp.tile([PB, QT], F32, tag="proj", name="q_ps")
                for c in range(4):
                    nc.tensor.matmul(
                        q_ps[:], wq2_sb[:, c, :],
                        xtf_sb[:, c, QT * t:QT * (t + 1)],
                        start=(c == 0), stop=(c == 3))
                nc.vector.tensor_copy(qt2[:, QT * t:QT * (t + 1)], q_ps[:])

            # ---- main loop over query tiles ----
            pend = None     # deferred PV group (keeps PE off ACT's tail)
            for t in range(NQT):
                if t % 2 == 0:
                    k_proj(t // 2)
                q_proj(t)
                v_proj(2 * t)
                v_proj(2 * t + 1)

                z_ps = zp.tile([E + 1, QT], F32, tag="z", name="z_ps")
                njb = 2 * t + 2
                groups = [list(range(g, min(g + GJ, njb)))
                          for g in range(0, njb, GJ)]
                for js in groups:
                    s_ps = sp.tile([PB, GJ * QT], F32, tag="s", name="s_ps")
                    for j in js:
                        sl = j - js[0]
                        half = slice(0, 64) if j % 2 == 0 else slice(64, 128)
                        nc.tensor.matmul(
                            s_ps[:, QT * sl:QT * (sl + 1)],
                            kt2[half, PB * j:PB * (j + 1)],
                            qt2[half, QT * t:QT * (t + 1)],
                            start=True, stop=True)

                    # flush previous group's deferred PV matmuls
                    if pend is not None:
                        _flush_pv(nc, pend)
                        pend = None

                    w = QT * len(js)
                    pt = ptp.tile([PB, GJ * QT], BF, tag="pt", name="pt")
                    nc.scalar.activation(pt[:, 0:w], s_ps[:, 0:w],
                                         mybir.ActivationFunctionType.Exp,
                                         scale=float(SCALE))
                    for j in js:
                        if j >= 2 * t:   # diagonal blocks: causal masks
                            sl = j - js[0]
                            mo = 384 if j == 2 * t else 128
                            nc.vector.tensor_mul(
                                pt[:, QT * sl:QT * (sl + 1)],
                                pt[:, QT * sl:QT * (sl + 1)],
                                msk_sb[:, mo:mo + QT])
                    pend = (z_ps, vext, pt, js, t)

                # attach Z evacuation of this tile to the last deferred group
                pend = pend + (zt, zsbp)

            # tail: flush last tile's PV + evacuation
            _flush_pv(nc, pend)

    nc.compile()
    return nc


def _flush_pv(nc, pend):
    """Emit the deferred PV matmul group (and Z evacuation if attached)."""
    z_ps, vext, pt, js, t = pend[:5]
    for j in js:
        sl = j - js[0]
        nc.tensor.matmul(
            z_ps[:],
            vext[:, (E + 1) * j:(E + 1) * (j + 1)],
            pt[:, QT * sl:QT * (sl + 1)],
            start=(j == 0), stop=(j == 2 * t + 1))
    if len(pend) > 5:
        zt, zsbp = pend[5], pend[6]
        z_sb = zsbp.tile([E + 1, QT], F32, tag="zsb", name="z_sb")
        nc.vector.tensor_copy(z_sb[:], z_ps[:])
        nc.sync.dma_start(zt[:, QT * t:QT * (t + 1)], z_sb[:])


def _get_nc():
    if "nc" not in _CACHE:
        _CACHE["nc"] = _build()
    return _CACHE["nc"]


def _host_inputs(X, Wq, Wk, Wv):
    """Per-core input maps. Core 2b+c: batch b, key parity c; X^T rotated
    left by 128*c columns."""
    w2 = lambda w: np.concatenate([w, w], axis=1).astype(BF16)
    wq2, wk2 = w2(Wq), w2(Wk)
    wv1 = Wv.astype(BF16)
    # mask master (same for both parities): msk[i, u] = 1 if i <= u - 384
    u = np.arange(896)[None, :]
    i = np.arange(PB)[:, None]
    msk = (i <= u - 384).astype(BF16)

    in_maps = []
    for b in range(B):
        xt = np.ascontiguousarray(np.asarray(X[b]).T).astype(BF16)
        for c in (0, 1):
            xtc = xt if c == 0 else np.ascontiguousarray(
                np.roll(xt, -PB * c, axis=1))
            in_maps.append({
                "xtf": xtc,
                "wq2": wq2, "wk2": wk2, "wv1": wv1, "msk": msk,
            })
    return in_maps


def _combine(results):
    Z = np.empty((B, S, E), np.float32)
    for b in range(B):
        za = results[2 * b]["zt"].astype(np.float32)
        zb = np.roll(results[2 * b + 1]["zt"].astype(np.float32),
                     PB, axis=1)     # un-rotate core B's query columns
        # B's wrapped query block (global q < 128) is garbage; A covers it.
        zb[:, 0:PB] = 0.0
        num = za[:E] + zb[:E]
        den = za[E] + zb[E]
        Z[b] = (num / den[None, :]).T
    return Z


def kernel(X, Wq, Wk, Wv, _trace=False, _tmpdir=None):
    from concourse.bass_utils import run_bass_kernel_spmd
    nc = _get_nc()
    in_maps = _host_inputs(X, Wq, Wk, Wv)
    kw = {}
    if _tmpdir is not None:
        kw["tmpdir"] = _tmpdir
    res = run_bass_kernel_spmd(nc, in_maps, core_ids=list(range(NCORES)),
                               trace=_trace, **kw)
    _CACHE["last"] = res
    return _combine(res.results)


# revision 11
# speedup vs baseline: 1.1921x; 1.0404x over previous
"""Causal single-head attention on 8 Trainium2 NeuronCores (Bass/Tile).

Problem: X[4,4096,512] fp32, Wq/Wk/Wv[512,64] fp32.
  Q=XWq, K=XWk, V=XWv ; Z = softmax(mask(QK^T)/8) V    -> [4,4096,64]

Sharding: 2 cores per batch, fully uniform SPMD program.
  - Keys/values are split by PARITY of 128-row key blocks: core A of a pair
    owns even key blocks, core B odd ones.  Each core's X^T input is
    ROTATED left by 128*parity columns by the host, which makes "my key
    blocks" sit at even 128-col positions for BOTH cores -- so one
    instruction stream with static addresses serves both.
  - Each core computes, for every query tile, partial attention over its
    own half of the keys with un-normalized softmax (no max subtraction --
    logits here are ~N(0, 0.2^2) so exp cannot overflow):
        numerator   N_c = sum_k exp(s)*V,   denominator D_c = sum_k exp(s)
    The host combines  Z = (N_A + N_B) / (D_A + D_B)  exactly.  The
    rotation wraps one query block on core B (tile 7); the host simply
    uses A-only partials for those 128 queries (A covers them fully).
  - Denominators come for free as column 64 of V_ext = [V | 1] in the
    P^T @ V_ext matmul.
  - Causality at 128-block granularity is structural (k-block count grows
    with the query tile); diagonal blocks are fixed by multiplying exp(S)
    by one of two static triangular masks (rotation makes the needed mask
    content identical on both cores).

On-chip dataflow (all matmuls bf16, fp32 PSUM accumulation):
  - scores are computed transposed  S^T[k,q] = K^T-block-stationary @ Q^T
    so P^T = exp(S^T) feeds the PV matmul with no on-chip transpose.
  - Q^T and K^T are produced doubled across the partition dim ([W|W]
    weights) so score matmuls (contraction=64) run 2x packed in the PE
    array via row groups (partitions 0-63 / 64-127).
  - V is produced in natural [k,64] layout by making the X^T chunk the
    stationary operand; K projection reads even 128-col blocks of X^T via
    a strided access pattern.
  - DMAs are split and ordered by first consumption; the PE starts ~11us
    in and the first exp fires ~14us in.
"""

import numpy as np
import ml_dtypes

import concourse.bacc as bacc
import concourse.bass as bass
import concourse.mybir as mybir
import concourse.tile as tile

B, S, DIN, E = 4, 4096, 512, 64
PB = 128            # partition / key block
QT = 512            # query tile width
NQT = S // QT       # 8 query tiles
NKB = S // PB       # 32 key blocks per batch
HKB = NKB // 2      # 16 packed key blocks per core
SH = S // 2         # 2048 packed keys per core
NCORES = 8
SCALE = 1.0 / np.sqrt(E)
GJ = 2              # k-blocks per exp group (PSUM banks = GJ)

BF16 = ml_dtypes.bfloat16
BF = mybir.dt.bfloat16
F32 = mybir.dt.float32

_CACHE = {}


def _build():
    nc = bacc.Bacc("TRN2", target_bir_lowering=False, debug=False,
                   enable_asserts=False, num_devices=NCORES)

    xtf_h = nc.dram_tensor("xtf", [DIN, S], BF, kind="ExternalInput")
    wq2_h = nc.dram_tensor("wq2", [DIN, 2 * E], BF, kind="ExternalInput")
    wk2_h = nc.dram_tensor("wk2", [DIN, 2 * E], BF, kind="ExternalInput")
    wv1_h = nc.dram_tensor("wv1", [DIN, E], BF, kind="ExternalInput")
    msk_h = nc.dram_tensor("msk", [PB, 896], BF, kind="ExternalInput")
    zt_h = nc.dram_tensor("zt", [E + 1, S], F32, kind="ExternalOutput")

    xtf_r = xtf_h.ap().rearrange("(c p) s -> p c s", p=PB)
    zt = zt_h.ap()

    with tile.TileContext(nc) as tc:
        with (
            tc.tile_pool(name="big", bufs=1) as big,
            tc.tile_pool(name="pt", bufs=4) as ptp,
            tc.tile_pool(name="zsb", bufs=2) as zsbp,
            tc.tile_pool(name="ppsum", bufs=2, space="PSUM") as pp,
            tc.tile_pool(name="spsum", bufs=2, space="PSUM") as sp,
            tc.tile_pool(name="zpsum", bufs=2, space="PSUM") as zp,
        ):
            # ---- persistent SBUF buffers ----
            xtf_sb = big.tile([PB, 4, S], BF, tag="xtf")
            wq2_sb = big.tile([PB, 4, 2 * E], BF, tag="wq2")
            wk2_sb = big.tile([PB, 4, 2 * E], BF, tag="wk2")
            wv1_sb = big.tile([PB, 4, E], BF, tag="wv1")
            msk_sb = big.tile([PB, 896], BF, tag="msk")
            qt2 = big.tile([PB, S], BF, tag="qt2")      # doubled Q^T (rot)
            kt2 = big.tile([PB, SH], BF, tag="kt2")     # doubled K^T (packed)
            vext = big.tile([PB, HKB * (E + 1)], BF, tag="vext")

            dma = nc.sync.dma_start

            # ---- input DMAs, ordered by first consumption ----
            dma(wk2_sb[:], wk2_h.ap().rearrange("(c p) m -> p c m", p=PB))
            dma(wq2_sb[:], wq2_h.ap().rearrange("(c p) m -> p c m", p=PB))
            dma(wv1_sb[:], wv1_h.ap().rearrange("(c p) m -> p c m", p=PB))
            dma(xtf_sb[:, :, 0:QT], xtf_r[:, :, 0:QT])
            dma(xtf_sb[:, :, QT:2 * QT], xtf_r[:, :, QT:2 * QT])
            dma(msk_sb[:], msk_h.ap())
            for pc in range(1, 4):     # remaining 1 MB pieces of X^T
                lo, hi = 2 * QT * pc, 2 * QT * (pc + 1)
                dma(xtf_sb[:, :, lo:hi], xtf_r[:, :, lo:hi])

            # ones columns of V_ext (V blocks overwrite cols 0..63 later)
            nc.vector.memset(vext[:], 1.0)

            def even_blocks(ap2d, s4):
                """[128, 512] strided view: even 128-col blocks
                {8s4, 8s4+2, 8s4+4, 8s4+6} of a [128, S] AP."""
                seg = ap2d[:, 1024 * s4:1024 * (s4 + 1)]
                return seg.rearrange("p (b two x) -> p b two x",
                                     two=2, x=PB)[:, :, 0, :]

            # projection chains; `other` interleaves a second accumulation
            # chain so consecutive matmuls hit different PSUM banks (hides
            # the PE drain that a same-bank accumulation chain exposes)
            def v_mm(v_ps, j, c):
                nc.tensor.matmul(
                    v_ps[:, 0:E], xtf_sb[:, c, 2 * PB * j:2 * PB * j + PB],
                    wv1_sb[:, c, :], start=(c == 0), stop=(c == 3))

            def v_evac(v_ps, j):
                nc.vector.tensor_copy(
                    vext[:, (E + 1) * j:(E + 1) * j + E], v_ps[:, 0:E])

            def k_proj(s4, vj=None):
                k_ps = pp.tile([PB, QT], F32, tag="proj", name="k_ps")
                v_ps = (pp.tile([PB, QT], F32, tag="proj", name="v_ps")
                        if vj is not None else None)
                for c in range(4):
                    nc.tensor.matmul(
                        k_ps[:], wk2_sb[:, c, :],
                        even_blocks(xtf_sb[:, c, :], s4),
                        start=(c == 0), stop=(c == 3))
                    if vj is not None:
                        v_mm(v_ps, vj, c)
                nc.vector.tensor_copy(kt2[:, QT * s4:QT * (s4 + 1)], k_ps[:])
                if vj is not None:
                    v_evac(v_ps, vj)

            def v_proj(j):
                v_ps = pp.tile([PB, QT], F32, tag="proj", name="v_ps")
                for c in range(4):
                    v_mm(v_ps, j, c)
                v_evac(v_ps, j)

            def q_proj(t, vj=None):
                q_ps = pp.tile([PB, QT], F32, tag="proj", name="q_ps")
                v_ps = (pp.tile([PB, QT], F32, tag="proj", name="v_ps")
                        if vj is not None else None)
                for c in range(4):
                    nc.tensor.matmul(
                        q_ps[:], wq2_sb[:, c, :],
                        xtf_sb[:, c, QT * t:QT * (t + 1)],
                        start=(c == 0), stop=(c == 3))
                    if vj is not None:
                        v_mm(v_ps, vj, c)
                nc.vector.tensor_copy(qt2[:, QT * t:QT * (t + 1)], q_ps[:])
                if vj is not None:
                    v_evac(v_ps, vj)

            # ---- main loop over query tiles ----
            pend = []       # deferred PV groups (keeps PE off ACT's tail)
            for t in range(NQT):
                q_proj(t, vj=2 * t)
                if t % 2 == 0:
                    k_proj(t // 2, vj=2 * t + 1)
                else:
                    v_proj(2 * t + 1)

                z_ps = zp.tile([E + 1, QT], F32, tag="z", name="z_ps")
                njb = 2 * t + 2
                groups = [list(range(g, min(g + GJ, njb)))
                          for g in range(0, njb, GJ)]
                for js in groups:
                    s_ps = sp.tile([PB, GJ * QT], F32, tag="s", name="s_ps")
                    for j in js:
                        sl = j - js[0]
                        half = slice(0, 64) if j % 2 == 0 else slice(64, 128)
                        nc.tensor.matmul(
                            s_ps[:, QT * sl:QT * (sl + 1)],
                            kt2[half, PB * j:PB * (j + 1)],
                            qt2[half, QT * t:QT * (t + 1)],
                            start=True, stop=True)

                    # flush deferred PV matmuls (keep up to 2 in flight)
                    if len(pend) >= 2:
                        _flush_pv(nc, pend.pop(0))

                    w = QT * len(js)
                    pt = ptp.tile([PB, GJ * QT], BF, tag="pt", name="pt")
                    nc.scalar.activation(pt[:, 0:w], s_ps[:, 0:w],
                                         mybir.ActivationFunctionType.Exp,
                                         scale=float(SCALE))
                    for j in js:
                        if j >= 2 * t:   # diagonal blocks: causal masks
                            sl = j - js[0]
                            mo = 384 if j == 2 * t else 128
                            nc.vector.tensor_mul(
                                pt[:, QT * sl:QT * (sl + 1)],
                                pt[:, QT * sl:QT * (sl + 1)],
                                msk_sb[:, mo:mo + QT])
                    pend.append((z_ps, vext, pt, js, t))

                # attach Z evacuation of this tile to the last deferred group
                pend[-1] = pend[-1] + (zt, zsbp)

            # tail: flush remaining deferred groups
            for p in pend:
                _flush_pv(nc, p)

    nc.compile()
    return nc


def _flush_pv(nc, pend):
    """Emit the deferred PV matmul group (and Z evacuation if attached)."""
    z_ps, vext, pt, js, t = pend[:5]
    for j in js:
        sl = j - js[0]
        nc.tensor.matmul(
            z_ps[:],
            vext[:, (E + 1) * j:(E + 1) * (j + 1)],
            pt[:, QT * sl:QT * (sl + 1)],
            start=(j == 0), stop=(j == 2 * t + 1))
    if len(pend) > 5:
        zt, zsbp = pend[5], pend[6]
        z_sb = zsbp.tile([E + 1, QT], F32, tag="zsb", name="z_sb")
        nc.vector.tensor_copy(z_sb[:], z_ps[:])
        nc.sync.dma_start(zt[:, QT * t:QT * (t + 1)], z_sb[:])


def _get_nc():
    if "nc" not in _CACHE:
        _CACHE["nc"] = _build()
    return _CACHE["nc"]


def _host_inputs(X, Wq, Wk, Wv):
    """Per-core input maps. Core 2b+c: batch b, key parity c; X^T rotated
    left by 128*c columns."""
    w2 = lambda w: np.concatenate([w, w], axis=1).astype(BF16)
    wq2, wk2 = w2(Wq), w2(Wk)
    wv1 = Wv.astype(BF16)
    # mask master (same for both parities): msk[i, u] = 1 if i <= u - 384
    u = np.arange(896)[None, :]
    i = np.arange(PB)[:, None]
    msk = (i <= u - 384).astype(BF16)

    in_maps = []
    for b in range(B):
        xt = np.ascontiguousarray(np.asarray(X[b]).T).astype(BF16)
        for c in (0, 1):
            xtc = xt if c == 0 else np.ascontiguousarray(
                np.roll(xt, -PB * c, axis=1))
            in_maps.append({
                "xtf": xtc,
                "wq2": wq2, "wk2": wk2, "wv1": wv1, "msk": msk,
            })
    return in_maps


def _combine(results):
    Z = np.empty((B, S, E), np.float32)
    for b in range(B):
        za = results[2 * b]["zt"].astype(np.float32)
        zb = np.roll(results[2 * b + 1]["zt"].astype(np.float32),
                     PB, axis=1)     # un-rotate core B's query columns
        # B's wrapped query block (global q < 128) is garbage; A covers it.
        zb[:, 0:PB] = 0.0
        num = za[:E] + zb[:E]
        den = za[E] + zb[E]
        Z[b] = (num / den[None, :]).T
    return Z


def kernel(X, Wq, Wk, Wv, _trace=False, _tmpdir=None):
    from concourse.bass_utils import run_bass_kernel_spmd
    nc = _get_nc()
    in_maps = _host_inputs(X, Wq, Wk, Wv)
    kw = {}
    if _tmpdir is not None:
        kw["tmpdir"] = _tmpdir
    res = run_bass_kernel_spmd(nc, in_maps, core_ids=list(range(NCORES)),
                               trace=_trace, **kw)
    _CACHE["last"] = res
    return _combine(res.results)


# revision 12
# speedup vs baseline: 1.2512x; 1.0496x over previous
"""Causal single-head attention on 8 Trainium2 NeuronCores (Bass/Tile).

Problem: X[4,4096,512] fp32, Wq/Wk/Wv[512,64] fp32.
  Q=XWq, K=XWk, V=XWv ; Z = softmax(mask(QK^T)/8) V    -> [4,4096,64]

Sharding: 2 cores per batch, fully uniform SPMD program.
  - Keys/values are split by PARITY of 128-row key blocks: core A of a pair
    owns even key blocks, core B odd ones.  Each core's X^T input is
    ROTATED left by 128*parity columns by the host, which makes "my key
    blocks" sit at even 128-col positions for BOTH cores -- so one
    instruction stream with static addresses serves both.
  - Each core computes, for every query tile, partial attention over its
    own half of the keys with un-normalized softmax (no max subtraction --
    logits here are ~N(0, 0.2^2) so exp cannot overflow):
        numerator   N_c = sum_k exp(s)*V,   denominator D_c = sum_k exp(s)
    The host combines  Z = (N_A + N_B) / (D_A + D_B)  exactly.  The
    rotation wraps one query block on core B (tile 7); the host simply
    uses A-only partials for those 128 queries (A covers them fully).
  - Denominators come for free as column 64 of V_ext = [V | 1] in the
    P^T @ V_ext matmul.
  - Causality at 128-block granularity is structural (k-block count grows
    with the query tile); diagonal blocks are fixed by multiplying exp(S)
    by one of two static triangular masks (rotation makes the needed mask
    content identical on both cores).

On-chip dataflow (all matmuls bf16, fp32 PSUM accumulation):
  - scores are computed transposed  S^T[k,q] = K^T-block-stationary @ Q^T
    so P^T = exp(S^T) feeds the PV matmul with no on-chip transpose.
  - Q^T and K^T are produced doubled across the partition dim ([W|W]
    weights) so score matmuls (contraction=64) run 2x packed in the PE
    array via row groups (partitions 0-63 / 64-127).
  - V is produced in natural [k,64] layout by making the X^T chunk the
    stationary operand; K projection reads even 128-col blocks of X^T via
    a strided access pattern.
  - DMAs are split and ordered by first consumption; the PE starts ~11us
    in and the first exp fires ~14us in.
"""

import numpy as np
import ml_dtypes

import concourse.bacc as bacc
import concourse.bass as bass
import concourse.mybir as mybir
import concourse.tile as tile

B, S, DIN, E = 4, 4096, 512, 64
PB = 128            # partition / key block
QT = 512            # query tile width
NQT = S // QT       # 8 query tiles
NKB = S // PB       # 32 key blocks per batch
HKB = NKB // 2      # 16 packed key blocks per core
SH = S // 2         # 2048 packed keys per core
NCORES = 8
SCALE = 1.0 / np.sqrt(E)
GJ = 2              # k-blocks per exp group (PSUM banks = GJ)

BF16 = ml_dtypes.bfloat16
BF = mybir.dt.bfloat16
F32 = mybir.dt.float32

_CACHE = {}


def _build():
    nc = bacc.Bacc("TRN2", target_bir_lowering=False, debug=False,
                   enable_asserts=False, num_devices=NCORES)

    xtf_h = nc.dram_tensor("xtf", [DIN, S], BF, kind="ExternalInput")
    wq2_h = nc.dram_tensor("wq2", [DIN, 2 * E], BF, kind="ExternalInput")
    wk2_h = nc.dram_tensor("wk2", [DIN, 2 * E], BF, kind="ExternalInput")
    wv1_h = nc.dram_tensor("wv1", [DIN, E], BF, kind="ExternalInput")
    msk_h = nc.dram_tensor("msk", [PB, 896], BF, kind="ExternalInput")
    zt_h = nc.dram_tensor("zt", [E + 1, S], F32, kind="ExternalOutput")

    xtf_r = xtf_h.ap().rearrange("(c p) s -> p c s", p=PB)
    zt = zt_h.ap()

    with tile.TileContext(nc) as tc:
        with (
            tc.tile_pool(name="big", bufs=1) as big,
            tc.tile_pool(name="pt", bufs=5) as ptp,
            tc.tile_pool(name="zsb", bufs=2) as zsbp,
            tc.tile_pool(name="ppsum", bufs=3, space="PSUM") as pp,
            tc.tile_pool(name="spsum", bufs=2, space="PSUM") as sp,
            tc.tile_pool(name="zpsum", bufs=1, space="PSUM") as zp,
        ):
            # ---- persistent SBUF buffers ----
            xtf_sb = big.tile([PB, 4, S], BF, tag="xtf")
            wq2_sb = big.tile([PB, 4, 2 * E], BF, tag="wq2")
            wk2_sb = big.tile([PB, 4, 2 * E], BF, tag="wk2")
            wv1_sb = big.tile([PB, 4, E], BF, tag="wv1")
            msk_sb = big.tile([PB, 896], BF, tag="msk")
            qt2 = big.tile([PB, S], BF, tag="qt2")      # doubled Q^T (rot)
            kt2 = big.tile([PB, SH], BF, tag="kt2")     # doubled K^T (packed)
            vext = big.tile([PB, HKB * (E + 1)], BF, tag="vext")

            dma = nc.sync.dma_start

            # ---- input DMAs, ordered by first consumption ----
            dma(wk2_sb[:], wk2_h.ap().rearrange("(c p) m -> p c m", p=PB))
            dma(wq2_sb[:], wq2_h.ap().rearrange("(c p) m -> p c m", p=PB))
            dma(wv1_sb[:], wv1_h.ap().rearrange("(c p) m -> p c m", p=PB))
            dma(xtf_sb[:, :, 0:QT], xtf_r[:, :, 0:QT])
            dma(xtf_sb[:, :, QT:2 * QT], xtf_r[:, :, QT:2 * QT])
            dma(msk_sb[:], msk_h.ap())
            for pc in range(1, 4):     # remaining 1 MB pieces of X^T
                lo, hi = 2 * QT * pc, 2 * QT * (pc + 1)
                dma(xtf_sb[:, :, lo:hi], xtf_r[:, :, lo:hi])

            # ones columns of V_ext (V blocks overwrite cols 0..63 later)
            nc.vector.memset(vext[:], 1.0)

            def even_blocks(ap2d, s4):
                """[128, 512] strided view: even 128-col blocks
                {8s4, 8s4+2, 8s4+4, 8s4+6} of a [128, S] AP."""
                seg = ap2d[:, 1024 * s4:1024 * (s4 + 1)]
                return seg.rearrange("p (b two x) -> p b two x",
                                     two=2, x=PB)[:, :, 0, :]

            # Projection chains.  Two q-tiles share each weight chunk
            # (so the 107ns LDWEIGHTS amortizes over two 213ns matmuls and
            # consecutive matmuls alternate PSUM banks, hiding the PE
            # drain); V-block chains ride along inside the long-matmul
            # streams so their weight loads hide under the 512-col matmuls.
            def v_mm(v_ps, j, c):
                nc.tensor.matmul(
                    v_ps[:, 0:E], xtf_sb[:, c, 2 * PB * j:2 * PB * j + PB],
                    wv1_sb[:, c, :], start=(c == 0), stop=(c == 3))

            def v_evac(v_ps, j):
                nc.vector.tensor_copy(
                    vext[:, (E + 1) * j:(E + 1) * j + E], v_ps[:, 0:E])

            def qq_v_proj(t, vj):
                # Q(t), Q(t+1) paired per weight chunk + V(vj) riding along
                qa = pp.tile([PB, QT], F32, tag="proj", name="qa_ps")
                qb = pp.tile([PB, QT], F32, tag="proj", name="qb_ps")
                v_ps = pp.tile([PB, QT], F32, tag="proj", name="v_ps")
                for c in range(4):
                    nc.tensor.matmul(
                        qa[:], wq2_sb[:, c, :],
                        xtf_sb[:, c, QT * t:QT * (t + 1)],
                        start=(c == 0), stop=(c == 3))
                    nc.tensor.matmul(
                        qb[:], wq2_sb[:, c, :],
                        xtf_sb[:, c, QT * (t + 1):QT * (t + 2)],
                        start=(c == 0), stop=(c == 3))
                    v_mm(v_ps, vj, c)
                nc.vector.tensor_copy(qt2[:, QT * t:QT * (t + 1)], qa[:])
                nc.vector.tensor_copy(qt2[:, QT * (t + 1):QT * (t + 2)], qb[:])
                v_evac(v_ps, vj)

            def k_v_proj(s4, vj):
                k_ps = pp.tile([PB, QT], F32, tag="proj", name="k_ps")
                v_ps = pp.tile([PB, QT], F32, tag="proj", name="v_ps")
                for c in range(4):
                    nc.tensor.matmul(
                        k_ps[:], wk2_sb[:, c, :],
                        even_blocks(xtf_sb[:, c, :], s4),
                        start=(c == 0), stop=(c == 3))
                    v_mm(v_ps, vj, c)
                nc.vector.tensor_copy(kt2[:, QT * s4:QT * (s4 + 1)], k_ps[:])
                v_evac(v_ps, vj)

            def v_proj(j):
                v_ps = pp.tile([PB, QT], F32, tag="proj", name="v_ps")
                for c in range(4):
                    v_mm(v_ps, j, c)
                v_evac(v_ps, j)

            # ---- main loop over query tiles ----
            pend = []       # deferred PV groups (keeps PE off ACT's tail)
            for t in range(NQT):
                if t % 2 == 0:
                    qq_v_proj(t, vj=2 * t)
                    k_v_proj(t // 2, vj=2 * t + 1)
                else:
                    v_proj(2 * t)
                    v_proj(2 * t + 1)

                z_ps = zp.tile([E + 1, QT], F32, tag="z", name="z_ps")
                njb = 2 * t + 2
                groups = [list(range(g, min(g + GJ, njb)))
                          for g in range(0, njb, GJ)]
                for js in groups:
                    s_ps = sp.tile([PB, GJ * QT], F32, tag="s", name="s_ps")
                    for j in js:
                        sl = j - js[0]
                        half = slice(0, 64) if j % 2 == 0 else slice(64, 128)
                        nc.tensor.matmul(
                            s_ps[:, QT * sl:QT * (sl + 1)],
                            kt2[half, PB * j:PB * (j + 1)],
                            qt2[half, QT * t:QT * (t + 1)],
                            start=True, stop=True)

                    # flush deferred PV matmuls (keep up to 3 in flight)
                    if len(pend) >= 3:
                        _flush_pv(nc, pend.pop(0))

                    w = QT * len(js)
                    pt = ptp.tile([PB, GJ * QT], BF, tag="pt", name="pt")
                    nc.scalar.activation(pt[:, 0:w], s_ps[:, 0:w],
                                         mybir.ActivationFunctionType.Exp,
                                         scale=float(SCALE))
                    for j in js:
                        if j >= 2 * t:   # diagonal blocks: causal masks
                            sl = j - js[0]
                            mo = 384 if j == 2 * t else 128
                            nc.vector.tensor_mul(
                                pt[:, QT * sl:QT * (sl + 1)],
                                pt[:, QT * sl:QT * (sl + 1)],
                                msk_sb[:, mo:mo + QT])
                    pend.append((z_ps, vext, pt, js, t))

                # attach Z evacuation of this tile to the last deferred group
                pend[-1] = pend[-1] + (zt, zsbp)

            # tail: flush remaining deferred groups
            for p in pend:
                _flush_pv(nc, p)

    nc.compile()
    return nc


def _flush_pv(nc, pend):
    """Emit the deferred PV matmul group (and Z evacuation if attached)."""
    z_ps, vext, pt, js, t = pend[:5]
    for j in js:
        sl = j - js[0]
        nc.tensor.matmul(
            z_ps[:],
            vext[:, (E + 1) * j:(E + 1) * (j + 1)],
            pt[:, QT * sl:QT * (sl + 1)],
            start=(j == 0), stop=(j == 2 * t + 1))
    if len(pend) > 5:
        zt, zsbp = pend[5], pend[6]
        z_sb = zsbp.tile([E + 1, QT], F32, tag="zsb", name="z_sb")
        nc.vector.tensor_copy(z_sb[:], z_ps[:])
        nc.sync.dma_start(zt[:, QT * t:QT * (t + 1)], z_sb[:])


def _get_nc():
    if "nc" not in _CACHE:
        _CACHE["nc"] = _build()
    return _CACHE["nc"]


def _host_inputs(X, Wq, Wk, Wv):
    """Per-core input maps. Core 2b+c: batch b, key parity c; X^T rotated
    left by 128*c columns."""
    w2 = lambda w: np.concatenate([w, w], axis=1).astype(BF16)
    wq2, wk2 = w2(Wq), w2(Wk)
    wv1 = Wv.astype(BF16)
    # mask master (same for both parities): msk[i, u] = 1 if i <= u - 384
    u = np.arange(896)[None, :]
    i = np.arange(PB)[:, None]
    msk = (i <= u - 384).astype(BF16)

    in_maps = []
    for b in range(B):
        xt = np.ascontiguousarray(np.asarray(X[b]).T).astype(BF16)
        for c in (0, 1):
            xtc = xt if c == 0 else np.ascontiguousarray(
                np.roll(xt, -PB * c, axis=1))
            in_maps.append({
                "xtf": xtc,
                "wq2": wq2, "wk2": wk2, "wv1": wv1, "msk": msk,
            })
    return in_maps


def _combine(results):
    Z = np.empty((B, S, E), np.float32)
    for b in range(B):
        za = results[2 * b]["zt"].astype(np.float32)
        zb = np.roll(results[2 * b + 1]["zt"].astype(np.float32),
                     PB, axis=1)     # un-rotate core B's query columns
        # B's wrapped query block (global q < 128) is garbage; A covers it.
        zb[:, 0:PB] = 0.0
        num = za[:E] + zb[:E]
        den = za[E] + zb[E]
        Z[b] = (num / den[None, :]).T
    return Z


def kernel(X, Wq, Wk, Wv, _trace=False, _tmpdir=None):
    from concourse.bass_utils import run_bass_kernel_spmd
    nc = _get_nc()
    in_maps = _host_inputs(X, Wq, Wk, Wv)
    kw = {}
    if _tmpdir is not None:
        kw["tmpdir"] = _tmpdir
    res = run_bass_kernel_spmd(nc, in_maps, core_ids=list(range(NCORES)),
                               trace=_trace, **kw)
    _CACHE["last"] = res
    return _combine(res.results)
